# revision 4
# baseline (speedup 1.0000x reference)
"""GAT-style graph attention kernel for Trainium2 (Bass/Tile), 8-core SPMD.

Per graph b (one NeuronCore each, B=8):
    X  = H[b] @ W                      [N, U]
    s  = X @ a_1   (per-query logit)   [N, 1]
    n  = X @ a_2   (per-key logit)     [N, 1]
    E  = leaky_relu(s_i + n_j, 0.2)    [N, N]
    P  = exp(E) * A[b]                 (== exp(E + NEG*(1-A)), A in {0,1})
    out= relu((P @ X) / rowsum(P))     [N, U]

v3 strategy (vs v2's two-frontend mix):
  Transport: 8-core microbenchmarks show ONE DMA queue with a deep
  buffer pool sustains ~375 GB/s while two concurrent queues interleave
  at packet granularity and drop to ~305.  So the whole A stream rides
  the gpsimd (SWDGE) queue as f32->f16-cast 2MiB singles with a
  12-buffer pool; W/a1/a2/H load f32 on the sync ring in the head
  (brief overlap only), and the outputs accumulate in SBUF and flush
  as one DMA at the very end.
  Compute: exp(leaky(s+n)) = max(exp(s+n), exp(0.2 s)*exp(0.2 n)).
  Per iteration: ONE ACT pass (x1 = Exp(n_bcast + s_i)), ONE fused DVE
  scalar_tensor_tensor pass (p = max(z_b * w_i, x1)), ONE DVE mask pass
  (p *= A, in place), plus the P^T PSUM->SBUF copies.  ACT ~3.7us,
  DVE ~3.6us, TensorE ~2.6us per iteration -- all under the ~5.6us
  DMA pace, so the kernel is memory-bound end to end and the compute
  tail after the last A byte is a single iteration's epilogue.
"""

import numpy as np
from contextlib import ExitStack

import concourse.bass as bass
import concourse.bacc as bacc
import concourse.mybir as mybir
import concourse.tile as tile
from concourse.masks import make_identity

F32 = mybir.dt.float32
F16 = mybir.dt.float16

N_NODES = 4096
N_FEAT = 128
N_UNITS = 64
N_CORES = 8
LEAKY_SLOPE = 0.2

A_BUFS = 12      # f16 A-tile pool depth (deep => SWDGE singles at line rate)
N_PRE = 4        # A loads issued before anything else on the gpsimd queue
FLUSH_SPLIT = False  # flush first half of outputs mid-stream


def build_nc(n_nodes=N_NODES):
    P = 128  # partitions
    U = N_UNITS
    F = N_FEAT
    n_t = n_nodes // P          # node tiles (32 full size)
    assert n_nodes % P == 0

    nc = bacc.Bacc(None)
    H_d = nc.declare_dram_parameter("H", [n_nodes, F], F32, isOutput=False)
    A_d = nc.declare_dram_parameter("A", [n_nodes, n_nodes], F32, isOutput=False)
    W_d = nc.declare_dram_parameter("W", [F, U], F32, isOutput=False)
    a1_d = nc.declare_dram_parameter("a_1", [U, 1], F32, isOutput=False)
    a2_d = nc.declare_dram_parameter("a_2", [U, 1], F32, isOutput=False)
    out_d = nc.declare_dram_parameter("out", [n_nodes, U], F32, isOutput=True)

    M = mybir.AluOpType
    AF = mybir.ActivationFunctionType

    with tile.TileContext(nc) as tc, ExitStack() as ctx:
        const = ctx.enter_context(tc.tile_pool(name="const", bufs=1))
        persist = ctx.enter_context(tc.tile_pool(name="persist", bufs=1))
        # A stream: deep f16 pool, one queue, strictly sequential issue.
        apool = ctx.enter_context(tc.tile_pool(name="apool", bufs=min(A_BUFS, n_t)))

        a_tiles = {}
        next_a = [0]

        def load_a():
            it = next_a[0]
            if it >= n_t:
                return
            next_a[0] = it + 1
            t = apool.tile([P, n_nodes], F16, tag="a16")
            nc.gpsimd.dma_start(t[:], A_d[it * P:(it + 1) * P, :])
            a_tiles[it] = t

        # First A loads head the gpsimd queue so SDMA saturates from t=0.
        for _ in range(min(N_PRE, n_t)):
            load_a()

        # Small weights ride the sync (HWDGE) ring; casts happen on ACT.
        W32 = const.tile([F, U], F32)
        nc.sync.dma_start(W32[:], W_d[:])
        a1_32 = const.tile([U, 1], F32)
        nc.sync.dma_start(a1_32[:], a1_d[:])
        a2_sb = const.tile([U, 1], F32)
        nc.sync.dma_start(a2_sb[:], a2_d[:])

        # identity (gpsimd ops; sit after the N_PRE A-load emissions)
        ident16 = const.tile([P, P], F16)
        make_identity(nc, ident16[:])

        W_sb = const.tile([F, U], F16)
        nc.scalar.copy(W_sb[:], W32[:])
        a1_sb = const.tile([U, 1], F16)
        nc.scalar.copy(a1_sb[:], a1_32[:])

        # a2 broadcast along free dim: a2b[u, c] = a2[u]
        a2b = const.tile([U, P], F16)
        nc.vector.memset(a2b[:], 1.0)
        nc.vector.tensor_scalar_mul(a2b[:], a2b[:], a2_sb[:, 0:1])

        # persistent per-graph tensors
        n_bcast = persist.tile([P, n_nodes], F32)     # n[j] bcast over partitions
        z_b = persist.tile([P, n_nodes], F16)         # exp(0.2 n[j]) bcast
        Xp_sb = persist.tile([P, n_t * (U + 1)], F16)  # X' tiles [X_t | 1]
        s_sb = persist.tile([P, n_t], F32)            # s column per query tile
        w_sb = persist.tile([P, n_t], F32)            # exp(0.2 s)
        dinv_sb = persist.tile([P, n_t], F32)
        outsbuf = persist.tile([P, n_t * U], F32)     # all outputs, flushed late
        nc.vector.memset(Xp_sb[:], 1.0)

        HCH = max(1, n_t // 4)

        # ---------------- prep: X, X^T, s, z_b, n_bcast ----------------
        with tc.tile_pool(name="hpool", bufs=1) as hpool, \
             tc.tile_pool(name="prep", bufs=6) as prep, \
             tc.tile_pool(name="prepx", bufs=1) as prepx, \
             tc.tile_pool(name="prep_ps", bufs=2, space="PSUM") as prep_ps, \
             tc.tile_pool(name="prep_ps1", bufs=2, space="PSUM") as prep_ps1:

            XT_sb = prepx.tile([U, n_nodes], F16)     # X^T (prep scope only)

            h_chunks = {}
            for c in range(0, n_t, HCH):
                hc32 = hpool.tile([P, HCH * F], F32, tag=f"h32_{c}")
                nc.sync.dma_start(
                    hc32[:].rearrange("p (t f) -> p t f", f=F),
                    H_d[c * P:(c + HCH) * P, :].rearrange(
                        "(t p) f -> p t f", p=P))
                hc = hpool.tile([P, HCH * F], F16, tag=f"h16_{c}")
                nc.scalar.copy(hc[:], hc32[:])
                h_chunks[c] = hc

            QB = 4 if n_t % 4 == 0 else 2
            s_tiles = {}
            for t2 in range(0, n_t, QB):
                hT_ps = prep_ps.tile([P, QB * P], F16, tag="hT_ps")
                for k in range(QB):
                    t = t2 + k
                    hc = h_chunks[(t // HCH) * HCH]
                    nc.tensor.transpose(hT_ps[:, k * P:k * P + F],
                                        hc[:, (t % HCH) * F:(t % HCH + 1) * F],
                                        ident16[:])
                hT_sb = prep.tile([F, QB * P], F16)
                nc.scalar.copy(hT_sb[:], hT_ps[:F, 0:QB * P])
                # X^T tiles: [U, node QB*128]
                xT_ps = prep_ps.tile([U, QB * P], F32, tag="xps")
                nc.tensor.matmul(xT_ps[:], W_sb[:], hT_sb[:], start=True, stop=True)
                if (t2 // QB) % 2 == 0:
                    nc.scalar.copy(XT_sb[:, t2 * P:(t2 + QB) * P], xT_ps[:])
                else:
                    nc.vector.tensor_copy(XT_sb[:, t2 * P:(t2 + QB) * P], xT_ps[:])
                # s[p, t] = (X @ a1)[t*128+p]
                s_q = prep_ps1.tile([P, QB], F32, tag="s_q")
                for k in range(QB):
                    nc.tensor.matmul(s_q[:, k:k + 1],
                                     XT_sb[:, (t2 + k) * P:(t2 + k + 1) * P],
                                     a1_sb[:], start=True, stop=True)
                s_sb_q = persist.tile([P, QB], F32, tag=f"s{t2}")
                nc.vector.tensor_copy(s_sb_q[:], s_q[:])
                s_tiles[t2] = s_sb_q
                nc.vector.tensor_copy(s_sb[:, t2:t2 + QB], s_q[:])
                # n_bcast[p, slice] = n[slice] broadcast over partitions
                nb_ps = prep_ps.tile([P, QB * P], F32, tag="nb_ps")
                nc.tensor.matmul(nb_ps[:], a2b[:],
                                 XT_sb[:, t2 * P:(t2 + QB) * P],
                                 start=True, stop=True)
                nc.vector.tensor_copy(n_bcast[:, t2 * P:(t2 + QB) * P],
                                      nb_ps[:])
                # z_b = exp(0.2 n) straight from PSUM on ACT
                nc.scalar.activation(z_b[:, t2 * P:(t2 + QB) * P], nb_ps[:],
                                     AF.Exp, scale=LEAKY_SLOPE)

            # X tiles for the H_cap matmuls, rebuilt from X^T off the
            # critical path (overlaps the start of the main loop).
            for t in range(n_t):
                x_ps = prep_ps.tile([P, U], F16, tag="xps")
                nc.tensor.transpose(x_ps[:, 0:U],
                                    XT_sb[:, t * P:(t + 1) * P],
                                    ident16[0:U, 0:U])
                nc.vector.tensor_copy(Xp_sb[:, t * (U + 1):t * (U + 1) + U],
                                      x_ps[:])
            # w = exp(0.2 s) per-partition scalars
            nc.scalar.activation(w_sb[:], s_sb[:], AF.Exp, scale=LEAKY_SLOPE)

        # ---------------- main loop over query tiles ----------------
        GROUP = 16                     # transposes per PSUM tile (2 banks)
        n_groups = (n_t + GROUP - 1) // GROUP
        LOOK = 2                       # produce lookahead (iters)

        with tc.tile_pool(name="x1pool", bufs=2) as x1pool, \
             tc.tile_pool(name="ppool", bufs=LOOK + 2) as ppool, \
             tc.tile_pool(name="ptpool", bufs=4) as ptpool, \
             tc.tile_pool(name="psT", bufs=3, space="PSUM") as psT, \
             tc.tile_pool(name="psAcc", bufs=2, space="PSUM") as psAcc:

            p_tiles = {}
            acc_tiles = {}

            def produce(it):
                load_a()               # keep the gpsimd queue fed, in order
                s_bias = s_tiles[(it // QB) * QB][:, it % QB:it % QB + 1]
                # x1 = exp(n_j + s_i): one ACT pass
                x1 = x1pool.tile([P, n_nodes], F16, tag="x1")
                nc.scalar.activation(x1[:], n_bcast[:], AF.Exp, bias=s_bias)
                # p = max(z_b * w_i, x1): one fused DVE pass
                p_t = ppool.tile([P, n_nodes], F16, tag="p")
                nc.vector.scalar_tensor_tensor(
                    p_t[:], z_b[:], w_sb[:, it:it + 1], x1[:],
                    M.mult, M.max)
                p_tiles[it] = p_t

            def consume(it):
                a_t = a_tiles.pop(it)
                p_t = p_tiles.pop(it)
                fine = it >= n_t - 2   # tail iterations: 8-block pipelining
                half = n_nodes // 2
                if not fine:
                    # mask in place, in halves so group transposes start earlier
                    for hf in range(2):
                        nc.vector.tensor_mul(
                            p_t[:, hf * half:(hf + 1) * half],
                            p_t[:, hf * half:(hf + 1) * half],
                            a_t[:, hf * half:(hf + 1) * half])

                # transpose P_m 128x128 blocks -> PSUM, copy groups to SBUF
                acc_ps = psAcc.tile([P, U + 1], F32, tag="acc_ps")
                for g in range(n_groups):
                    k_n = min(GROUP, n_t - g * GROUP)
                    pt_ps = psT.tile([P, GROUP * P], F16, tag="pt_ps")
                    for half_g in range(2 if fine else 1):
                        if fine:
                            lo = g * GROUP * P + half_g * (GROUP // 2) * P
                            hi = lo + (GROUP // 2) * P
                            nc.vector.tensor_mul(p_t[:, lo:hi], p_t[:, lo:hi],
                                                 a_t[:, lo:hi])
                            ks = range(half_g * (GROUP // 2),
                                       min(k_n, (half_g + 1) * (GROUP // 2)))
                        else:
                            ks = range(k_n)
                        for k in ks:
                            jt = g * GROUP + k
                            nc.tensor.transpose(pt_ps[:, k * P:(k + 1) * P],
                                                p_t[:, jt * P:(jt + 1) * P],
                                                ident16[:])
                    pt_sb = ptpool.tile([P, GROUP * P], F16, tag="pt_sb")
                    w_n = k_n * P
                    if fine:
                        # split the copy across both engines in the tail
                        nc.scalar.copy(pt_sb[:, 0:w_n // 2], pt_ps[:, 0:w_n // 2])
                        nc.vector.tensor_copy(pt_sb[:, w_n // 2:w_n],
                                              pt_ps[:, w_n // 2:w_n])
                    else:
                        nc.vector.tensor_copy(pt_sb[:, 0:w_n], pt_ps[:, 0:w_n])
                    # H_cap accumulation for this group's j tiles
                    for k in range(k_n):
                        jt = g * GROUP + k
                        nc.tensor.matmul(
                            acc_ps[:], pt_sb[:, k * P:(k + 1) * P],
                            Xp_sb[:, jt * (U + 1):(jt + 1) * (U + 1)],
                            start=(jt == 0), stop=(jt == n_t - 1))

                nc.vector.reciprocal(dinv_sb[:, it:it + 1], acc_ps[:, U:U + 1])
                acc_tiles[it] = acc_ps

            def emit_out(it):
                # out = relu(H_cap[:, :U] / H_cap[:, U]) -- relu+scale on ACT,
                # into the SBUF output buffer (flushed by DMA at the end).
                acc_ps = acc_tiles.pop(it)
                nc.scalar.activation(outsbuf[:, it * U:(it + 1) * U],
                                     acc_ps[:, 0:U], AF.Relu,
                                     scale=dinv_sb[:, it:it + 1])

            for it in range(n_t + LOOK + 1):
                if it < n_t:
                    produce(it)
                if LOOK <= it < n_t + LOOK:
                    ct = it - LOOK
                    consume(ct)
                    if ct >= n_t - 2:
                        emit_out(ct)
                if LOOK < it < n_t + LOOK - 1:
                    emit_out(it - LOOK - 1)
                if FLUSH_SPLIT and it == n_t * 5 // 8:
                    hn = n_t // 2
                    nc.sync.dma_start(
                        out_d[0:hn * P, :].rearrange("(t p) u -> p t u", p=P),
                        outsbuf[:, 0:hn * U].rearrange("p (t u) -> p t u", u=U))

            # final output flush on the idle sync ring
            lo = (n_t // 2) if FLUSH_SPLIT else 0
            nc.sync.dma_start(
                out_d[lo * P:n_t * P, :].rearrange("(t p) u -> p t u", p=P),
                outsbuf[:, lo * U:n_t * U].rearrange("p (t u) -> p t u", u=U))

    nc.compile()
    return nc


_NC_CACHE = {}


def _get_nc(n_nodes=N_NODES):
    if n_nodes not in _NC_CACHE:
        _NC_CACHE[n_nodes] = build_nc(n_nodes)
    return _NC_CACHE[n_nodes]


def kernel(H, A, W, a_1, a_2):
    """Full inputs in, full output out. Shards batch across 8 NeuronCores."""
    import os
    # The axon trace path needs antenv.axon_hooks, which this image lacks;
    # make sure an inherited BASS_TRACE can't route us there.
    os.environ["BASS_NEVER_TRACE"] = "1"
    from concourse.bass_utils import run_bass_kernel_spmd

    B = H.shape[0]
    assert B == N_CORES
    nc = _get_nc(H.shape[1])
    in_maps = [
        {
            "H": np.ascontiguousarray(H[b], dtype=np.float32),
            "A": np.ascontiguousarray(A[b], dtype=np.float32),
            "W": np.ascontiguousarray(W, dtype=np.float32),
            "a_1": np.ascontiguousarray(a_1, dtype=np.float32),
            "a_2": np.ascontiguousarray(a_2, dtype=np.float32),
        }
        for b in range(B)
    ]
    res = run_bass_kernel_spmd(nc, in_maps, core_ids=list(range(N_CORES)))
    out = np.stack([res.results[b]["out"] for b in range(B)]).astype(np.float32)
    return out


# revision 5
# speedup vs baseline: 1.1762x; 1.1762x over previous
"""GAT-style graph attention kernel for Trainium2 (Bass/Tile), 8-core SPMD.

Per graph b (one NeuronCore each, B=8):
    X  = H[b] @ W                      [N, U]
    s  = X @ a_1   (per-query logit)   [N, 1]
    n  = X @ a_2   (per-key logit)     [N, 1]
    E  = leaky_relu(s_i + n_j, 0.2)    [N, N]
    P  = exp(E) * A[b]                 (== exp(E + NEG*(1-A)), A in {0,1})
    out= relu((P @ X) / rowsum(P))     [N, U]

v3 strategy (vs v2's two-frontend mix):
  Transport: 8-core microbenchmarks show ONE DMA queue with a deep
  buffer pool sustains ~375 GB/s while two concurrent queues interleave
  at packet granularity and drop to ~305.  So the whole A stream rides
  the gpsimd (SWDGE) queue as f32->f16-cast 2MiB singles with a
  12-buffer pool; W/a1/a2/H load f32 on the sync ring in the head
  (brief overlap only), and the outputs accumulate in SBUF and flush
  as one DMA at the very end.
  Compute: exp(leaky(s+n)) = max(exp(s+n), exp(0.2 s)*exp(0.2 n)).
  Per iteration: ONE ACT pass (x1 = Exp(n_bcast + s_i)), ONE fused DVE
  scalar_tensor_tensor pass (p = max(z_b * w_i, x1)), ONE DVE mask pass
  (p *= A, in place), plus the P^T PSUM->SBUF copies.  ACT ~3.7us,
  DVE ~3.6us, TensorE ~2.6us per iteration -- all under the ~5.6us
  DMA pace, so the kernel is memory-bound end to end and the compute
  tail after the last A byte is a single iteration's epilogue.
"""

import numpy as np
from contextlib import ExitStack

import concourse.bass as bass
import concourse.bacc as bacc
import concourse.mybir as mybir
import concourse.tile as tile
from concourse.masks import make_identity

F32 = mybir.dt.float32
F16 = mybir.dt.float16

N_NODES = 4096
N_FEAT = 128
N_UNITS = 64
N_CORES = 8
LEAKY_SLOPE = 0.2

A_BUFS = 11      # f16 A-tile pool depth (deep => SWDGE singles at line rate)
N_PRE = 4        # A loads issued before anything else on the gpsimd queue
FLUSH_SPLIT = False  # flush first half of outputs mid-stream


def build_nc(n_nodes=N_NODES):
    P = 128  # partitions
    U = N_UNITS
    F = N_FEAT
    n_t = n_nodes // P          # node tiles (32 full size)
    assert n_nodes % P == 0

    nc = bacc.Bacc(None)
    H_d = nc.declare_dram_parameter("H", [n_nodes, F], F32, isOutput=False)
    A_d = nc.declare_dram_parameter("A", [n_nodes, n_nodes], F32, isOutput=False)
    W_d = nc.declare_dram_parameter("W", [F, U], F32, isOutput=False)
    a1_d = nc.declare_dram_parameter("a_1", [U, 1], F32, isOutput=False)
    a2_d = nc.declare_dram_parameter("a_2", [U, 1], F32, isOutput=False)
    out_d = nc.declare_dram_parameter("out", [n_nodes, U], F32, isOutput=True)

    M = mybir.AluOpType
    AF = mybir.ActivationFunctionType

    with tile.TileContext(nc) as tc, ExitStack() as ctx:
        const = ctx.enter_context(tc.tile_pool(name="const", bufs=1))
        persist = ctx.enter_context(tc.tile_pool(name="persist", bufs=1))
        # A stream: deep f16 pool, one queue, strictly sequential issue.
        apool = ctx.enter_context(tc.tile_pool(name="apool", bufs=min(A_BUFS, n_t)))

        a_tiles = {}
        next_a = [0]

        def load_a():
            it = next_a[0]
            if it >= n_t:
                return
            next_a[0] = it + 1
            t = apool.tile([P, n_nodes], F16, tag="a16")
            nc.gpsimd.dma_start(t[:], A_d[it * P:(it + 1) * P, :])
            a_tiles[it] = t

        # Small weights + H ride the gpsimd queue (f32->f16 cast) AHEAD of
        # the A singles; a2 (kept f32) rides sync.
        W_sb = const.tile([F, U], F16)
        nc.gpsimd.dma_start(W_sb[:], W_d[:])
        a1_sb = const.tile([U, 1], F16)
        nc.gpsimd.dma_start(a1_sb[:], a1_d[:])
        a2_sb = const.tile([U, 1], F32)
        nc.sync.dma_start(a2_sb[:], a2_d[:])

        ident16 = const.tile([P, P], F16)

        # a2 broadcast along free dim: a2b[u, c] = a2[u]
        a2b = const.tile([U, P], F16)
        nc.vector.memset(a2b[:], 1.0)
        nc.vector.tensor_scalar_mul(a2b[:], a2b[:], a2_sb[:, 0:1])

        # persistent per-graph tensors
        n_bcast = persist.tile([P, n_nodes], F32)     # n[j] bcast over partitions
        z_b = persist.tile([P, n_nodes], F16)         # exp(0.2 n[j]) bcast
        Xp_sb = persist.tile([P, n_t * (U + 1)], F16)  # X' tiles [X_t | 1]
        s_sb = persist.tile([P, n_t], F32)            # s column per query tile
        w_sb = persist.tile([P, n_t], F32)            # exp(0.2 s)
        dinv_sb = persist.tile([P, n_t], F32)
        outsbuf = persist.tile([P, n_t * U], F32)     # all outputs, flushed late
        nc.vector.memset(Xp_sb[:], 1.0)

        HCH = max(1, n_t // 4)

        # ---------------- prep: X, X^T, s, z_b, n_bcast ----------------
        with tc.tile_pool(name="hpool", bufs=1) as hpool, \
             tc.tile_pool(name="prep", bufs=6) as prep, \
             tc.tile_pool(name="prepx", bufs=1) as prepx, \
             tc.tile_pool(name="prep_ps", bufs=2, space="PSUM") as prep_ps, \
             tc.tile_pool(name="prep_ps1", bufs=2, space="PSUM") as prep_ps1:

            XT_sb = prepx.tile([U, n_nodes], F16)     # X^T (prep scope only)

            h_chunks = {}
            for c in range(0, n_t, HCH):
                hc = hpool.tile([P, HCH * F], F16, tag=f"h16_{c}")
                nc.gpsimd.dma_start(
                    hc[:].rearrange("p (t f) -> p t f", f=F),
                    H_d[c * P:(c + HCH) * P, :].rearrange(
                        "(t p) f -> p t f", p=P))
                h_chunks[c] = hc

            # identity on gpsimd compute, then the first A emissions
            make_identity(nc, ident16[:])
            for _ in range(min(N_PRE, n_t)):
                load_a()

            QB = 4 if n_t % 4 == 0 else 2
            s_tiles = {}
            for t2 in range(0, n_t, QB):
                hT_ps = prep_ps.tile([P, QB * P], F16, tag="hT_ps")
                for k in range(QB):
                    t = t2 + k
                    hc = h_chunks[(t // HCH) * HCH]
                    nc.tensor.transpose(hT_ps[:, k * P:k * P + F],
                                        hc[:, (t % HCH) * F:(t % HCH + 1) * F],
                                        ident16[:])
                hT_sb = prep.tile([F, QB * P], F16)
                nc.scalar.copy(hT_sb[:], hT_ps[:F, 0:QB * P])
                # X^T tiles: [U, node QB*128]
                xT_ps = prep_ps.tile([U, QB * P], F32, tag="xps")
                nc.tensor.matmul(xT_ps[:], W_sb[:], hT_sb[:], start=True, stop=True)
                if (t2 // QB) % 2 == 0:
                    nc.scalar.copy(XT_sb[:, t2 * P:(t2 + QB) * P], xT_ps[:])
                else:
                    nc.vector.tensor_copy(XT_sb[:, t2 * P:(t2 + QB) * P], xT_ps[:])
                # s[p, t] = (X @ a1)[t*128+p]
                s_q = prep_ps1.tile([P, QB], F32, tag="s_q")
                for k in range(QB):
                    nc.tensor.matmul(s_q[:, k:k + 1],
                                     XT_sb[:, (t2 + k) * P:(t2 + k + 1) * P],
                                     a1_sb[:], start=True, stop=True)
                s_sb_q = persist.tile([P, QB], F32, tag=f"s{t2}")
                nc.vector.tensor_copy(s_sb_q[:], s_q[:])
                s_tiles[t2] = s_sb_q
                nc.vector.tensor_copy(s_sb[:, t2:t2 + QB], s_q[:])
                # n_bcast[p, slice] = n[slice] broadcast over partitions
                nb_ps = prep_ps.tile([P, QB * P], F32, tag="nb_ps")
                nc.tensor.matmul(nb_ps[:], a2b[:],
                                 XT_sb[:, t2 * P:(t2 + QB) * P],
                                 start=True, stop=True)
                nc.vector.tensor_copy(n_bcast[:, t2 * P:(t2 + QB) * P],
                                      nb_ps[:])
                # z_b = exp(0.2 n) straight from PSUM on ACT
                nc.scalar.activation(z_b[:, t2 * P:(t2 + QB) * P], nb_ps[:],
                                     AF.Exp, scale=LEAKY_SLOPE)

            # X tiles for the H_cap matmuls, rebuilt from X^T off the
            # critical path (overlaps the start of the main loop).
            for t in range(n_t):
                x_ps = prep_ps.tile([P, U], F16, tag="xps")
                nc.tensor.transpose(x_ps[:, 0:U],
                                    XT_sb[:, t * P:(t + 1) * P],
                                    ident16[0:U, 0:U])
                nc.vector.tensor_copy(Xp_sb[:, t * (U + 1):t * (U + 1) + U],
                                      x_ps[:])
            # w = exp(0.2 s) per-partition scalars
            nc.scalar.activation(w_sb[:], s_sb[:], AF.Exp, scale=LEAKY_SLOPE)

        # ---------------- main loop over query tiles ----------------
        GROUP = 16                     # transposes per PSUM tile (2 banks)
        n_groups = (n_t + GROUP - 1) // GROUP
        LOOK = 2                       # produce lookahead (iters)

        with tc.tile_pool(name="x1pool", bufs=2) as x1pool, \
             tc.tile_pool(name="x2pool", bufs=1) as x2pool, \
             tc.tile_pool(name="ppool", bufs=LOOK + 2) as ppool, \
             tc.tile_pool(name="ptpool", bufs=4) as ptpool, \
             tc.tile_pool(name="psT", bufs=3, space="PSUM") as psT, \
             tc.tile_pool(name="psAcc", bufs=2, space="PSUM") as psAcc:

            p_tiles = {}
            acc_tiles = {}

            def produce(it):
                load_a()               # keep the gpsimd queue fed, in order
                s_bias = s_tiles[(it // QB) * QB][:, it % QB:it % QB + 1]
                # x1 = exp(n_j + s_i): one ACT pass
                x1 = x1pool.tile([P, n_nodes], F16, tag="x1")
                nc.scalar.activation(x1[:], n_bcast[:], AF.Exp, bias=s_bias)
                # x2 = z_b * w_i ; p = max(x1, x2)  (two DVE f16 passes)
                x2 = x2pool.tile([P, n_nodes], F16, tag="x2")
                nc.vector.tensor_scalar_mul(x2[:], z_b[:], w_sb[:, it:it + 1])
                p_t = ppool.tile([P, n_nodes], F16, tag="p")
                nc.vector.tensor_max(p_t[:], x1[:], x2[:])
                p_tiles[it] = p_t

            def consume(it):
                a_t = a_tiles.pop(it)
                p_t = p_tiles.pop(it)
                fine = it >= n_t - 2   # tail iterations: 8-block pipelining
                half = n_nodes // 2
                if not fine:
                    # mask in place on GpSimd (DVE is busier), in halves so
                    # group transposes start earlier
                    for hf in range(2):
                        nc.gpsimd.tensor_mul(
                            p_t[:, hf * half:(hf + 1) * half],
                            p_t[:, hf * half:(hf + 1) * half],
                            a_t[:, hf * half:(hf + 1) * half])

                # transpose P_m 128x128 blocks -> PSUM, copy groups to SBUF
                acc_ps = psAcc.tile([P, U + 1], F32, tag="acc_ps")
                for g in range(n_groups):
                    k_n = min(GROUP, n_t - g * GROUP)
                    pt_ps = psT.tile([P, GROUP * P], F16, tag="pt_ps")
                    for half_g in range(2 if fine else 1):
                        if fine:
                            lo = g * GROUP * P + half_g * (GROUP // 2) * P
                            hi = lo + (GROUP // 2) * P
                            nc.vector.tensor_mul(p_t[:, lo:hi], p_t[:, lo:hi],
                                                 a_t[:, lo:hi])
                            ks = range(half_g * (GROUP // 2),
                                       min(k_n, (half_g + 1) * (GROUP // 2)))
                        else:
                            ks = range(k_n)
                        for k in ks:
                            jt = g * GROUP + k
                            nc.tensor.transpose(pt_ps[:, k * P:(k + 1) * P],
                                                p_t[:, jt * P:(jt + 1) * P],
                                                ident16[:])
                    pt_sb = ptpool.tile([P, GROUP * P], F16, tag="pt_sb")
                    w_n = k_n * P
                    if fine:
                        # split the copy across both engines in the tail
                        nc.scalar.copy(pt_sb[:, 0:w_n // 2], pt_ps[:, 0:w_n // 2])
                        nc.vector.tensor_copy(pt_sb[:, w_n // 2:w_n],
                                              pt_ps[:, w_n // 2:w_n])
                    else:
                        nc.vector.tensor_copy(pt_sb[:, 0:w_n], pt_ps[:, 0:w_n])
                    # H_cap accumulation for this group's j tiles
                    for k in range(k_n):
                        jt = g * GROUP + k
                        nc.tensor.matmul(
                            acc_ps[:], pt_sb[:, k * P:(k + 1) * P],
                            Xp_sb[:, jt * (U + 1):(jt + 1) * (U + 1)],
                            start=(jt == 0), stop=(jt == n_t - 1))

                nc.vector.reciprocal(dinv_sb[:, it:it + 1], acc_ps[:, U:U + 1])
                acc_tiles[it] = acc_ps

            def emit_out(it):
                # out = relu(H_cap[:, :U] / H_cap[:, U]) -- relu+scale on ACT,
                # into the SBUF output buffer (flushed by DMA at the end).
                acc_ps = acc_tiles.pop(it)
                nc.scalar.activation(outsbuf[:, it * U:(it + 1) * U],
                                     acc_ps[:, 0:U], AF.Relu,
                                     scale=dinv_sb[:, it:it + 1])

            for it in range(n_t + LOOK + 1):
                if it < n_t:
                    produce(it)
                if LOOK <= it < n_t + LOOK:
                    ct = it - LOOK
                    consume(ct)
                    if ct >= n_t - 2:
                        emit_out(ct)
                if LOOK < it < n_t + LOOK - 1:
                    emit_out(it - LOOK - 1)
                if FLUSH_SPLIT and it == n_t * 5 // 8:
                    hn = n_t // 2
                    nc.sync.dma_start(
                        out_d[0:hn * P, :].rearrange("(t p) u -> p t u", p=P),
                        outsbuf[:, 0:hn * U].rearrange("p (t u) -> p t u", u=U))

            # final output flush on the idle sync ring
            lo = (n_t // 2) if FLUSH_SPLIT else 0
            nc.sync.dma_start(
                out_d[lo * P:n_t * P, :].rearrange("(t p) u -> p t u", p=P),
                outsbuf[:, lo * U:n_t * U].rearrange("p (t u) -> p t u", u=U))

    nc.compile()
    return nc


_NC_CACHE = {}


def _get_nc(n_nodes=N_NODES):
    if n_nodes not in _NC_CACHE:
        _NC_CACHE[n_nodes] = build_nc(n_nodes)
    return _NC_CACHE[n_nodes]


def kernel(H, A, W, a_1, a_2):
    """Full inputs in, full output out. Shards batch across 8 NeuronCores."""
    import os
    # The axon trace path needs antenv.axon_hooks, which this image lacks;
    # make sure an inherited BASS_TRACE can't route us there.
    os.environ["BASS_NEVER_TRACE"] = "1"
    from concourse.bass_utils import run_bass_kernel_spmd

    B = H.shape[0]
    assert B == N_CORES
    nc = _get_nc(H.shape[1])
    in_maps = [
        {
            "H": np.ascontiguousarray(H[b], dtype=np.float32),
            "A": np.ascontiguousarray(A[b], dtype=np.float32),
            "W": np.ascontiguousarray(W, dtype=np.float32),
            "a_1": np.ascontiguousarray(a_1, dtype=np.float32),
            "a_2": np.ascontiguousarray(a_2, dtype=np.float32),
        }
        for b in range(B)
    ]
    res = run_bass_kernel_spmd(nc, in_maps, core_ids=list(range(N_CORES)))
    out = np.stack([res.results[b]["out"] for b in range(B)]).astype(np.float32)
    return out


# revision 6
# speedup vs baseline: 1.6026x; 1.3625x over previous
"""GAT-style graph attention kernel for Trainium2 (Bass/Tile), 8-core SPMD.

Per graph b (one NeuronCore each, B=8):
    X  = H[b] @ W                      [N, U]
    s  = X @ a_1   (per-query logit)   [N, 1]
    n  = X @ a_2   (per-key logit)     [N, 1]
    E  = leaky_relu(s_i + n_j, 0.2)    [N, N]
    P  = exp(E) * A[b]                 (== exp(E + NEG*(1-A)), A in {0,1})
    out= relu((P @ X) / rowsum(P))     [N, U]

v3 strategy (vs v2's two-frontend mix):
  Transport: 8-core microbenchmarks show ONE DMA queue with a deep
  buffer pool sustains ~375 GB/s while two concurrent queues interleave
  at packet granularity and drop to ~305.  So the whole A stream rides
  the gpsimd (SWDGE) queue as f32->f16-cast 2MiB singles with a
  12-buffer pool; W/a1/a2/H load f32 on the sync ring in the head
  (brief overlap only), and the outputs accumulate in SBUF and flush
  as one DMA at the very end.
  Compute: exp(leaky(s+n)) = max(exp(s+n), exp(0.2 s)*exp(0.2 n)).
  Per iteration: ONE ACT pass (x1 = Exp(n_bcast + s_i)), ONE fused DVE
  scalar_tensor_tensor pass (p = max(z_b * w_i, x1)), ONE DVE mask pass
  (p *= A, in place), plus the P^T PSUM->SBUF copies.  ACT ~3.7us,
  DVE ~3.6us, TensorE ~2.6us per iteration -- all under the ~5.6us
  DMA pace, so the kernel is memory-bound end to end and the compute
  tail after the last A byte is a single iteration's epilogue.
"""

import numpy as np
from contextlib import ExitStack

import concourse.bass as bass
import concourse.bacc as bacc
import concourse.mybir as mybir
import concourse.tile as tile
from concourse.masks import make_identity

F32 = mybir.dt.float32
F16 = mybir.dt.float16

N_NODES = 4096
N_FEAT = 128
N_UNITS = 64
N_CORES = 8
LEAKY_SLOPE = 0.2

A_BUFS = 11      # f16 A-tile pool depth (deep => SWDGE singles at line rate)
# ACT-heavy (Prelu+Exp, no DVE) iterations; the rest are DVE-heavy
# (Exp + tensor_scalar + max).  12:20 balances ACT ~5.3us vs DVE ~5.3us.
P1_ITERS = (0, 1, 2, 3, 5, 8, 11, 14, 17, 20, 23, 26)
N_PRE = 4        # A loads issued before anything else on the gpsimd queue
FLUSH_SPLIT = False  # flush first half of outputs mid-stream


def build_nc(n_nodes=N_NODES):
    P = 128  # partitions
    U = N_UNITS
    F = N_FEAT
    n_t = n_nodes // P          # node tiles (32 full size)
    assert n_nodes % P == 0

    nc = bacc.Bacc(None)
    H_d = nc.declare_dram_parameter("H", [n_nodes, F], F32, isOutput=False)
    A_d = nc.declare_dram_parameter("A", [n_nodes, n_nodes], F32, isOutput=False)
    W_d = nc.declare_dram_parameter("W", [F, U], F32, isOutput=False)
    a1_d = nc.declare_dram_parameter("a_1", [U, 1], F32, isOutput=False)
    a2_d = nc.declare_dram_parameter("a_2", [U, 1], F32, isOutput=False)
    out_d = nc.declare_dram_parameter("out", [n_nodes, U], F32, isOutput=True)

    M = mybir.AluOpType
    AF = mybir.ActivationFunctionType

    with tile.TileContext(nc) as tc, ExitStack() as ctx:
        const = ctx.enter_context(tc.tile_pool(name="const", bufs=1))
        persist = ctx.enter_context(tc.tile_pool(name="persist", bufs=1))
        # A stream: deep f16 pool, one queue, strictly sequential issue.
        apool = ctx.enter_context(tc.tile_pool(name="apool", bufs=min(A_BUFS, n_t)))

        a_tiles = {}
        next_a = [0]

        def load_a():
            it = next_a[0]
            if it >= n_t:
                return
            next_a[0] = it + 1
            t = apool.tile([P, n_nodes], F16, tag="a16")
            nc.gpsimd.dma_start(t[:], A_d[it * P:(it + 1) * P, :])
            a_tiles[it] = t

        # Small weights + H ride the gpsimd queue (f32->f16 cast) AHEAD of
        # the A singles; a2 (kept f32) rides sync.
        W_sb = const.tile([F, U], F16)
        nc.gpsimd.dma_start(W_sb[:], W_d[:])
        a1_sb = const.tile([U, 1], F16)
        nc.gpsimd.dma_start(a1_sb[:], a1_d[:])
        a2_sb = const.tile([U, 1], F32)
        nc.sync.dma_start(a2_sb[:], a2_d[:])

        ident16 = const.tile([P, P], F16)

        # a2 broadcast along free dim: a2b[u, c] = a2[u]
        a2b = const.tile([U, P], F16)
        nc.vector.memset(a2b[:], 1.0)
        nc.vector.tensor_scalar_mul(a2b[:], a2b[:], a2_sb[:, 0:1])

        # persistent per-graph tensors
        n_bcast = persist.tile([P, n_nodes], F32)     # n[j] bcast over partitions
        z_b = persist.tile([P, n_nodes], F16)         # exp(0.2 n[j]) bcast
        Xp_sb = persist.tile([P, n_t * (U + 1)], F16)  # X' tiles [X_t | 1]
        s_sb = persist.tile([P, n_t], F32)            # s column per query tile
        w_sb = persist.tile([P, n_t], F32)            # exp(0.2 s)
        dinv_sb = persist.tile([P, n_t], F32)
        outsbuf = persist.tile([P, n_t * U], F32)     # all outputs, flushed late
        nc.vector.memset(Xp_sb[:], 1.0)

        HCH = max(1, n_t // 4)

        # ---------------- prep: X, X^T, s, z_b, n_bcast ----------------
        with tc.tile_pool(name="hpool", bufs=1) as hpool, \
             tc.tile_pool(name="prep", bufs=6) as prep, \
             tc.tile_pool(name="prepx", bufs=1) as prepx, \
             tc.tile_pool(name="prep_ps", bufs=2, space="PSUM") as prep_ps, \
             tc.tile_pool(name="prep_ps1", bufs=2, space="PSUM") as prep_ps1:

            XT_sb = prepx.tile([U, n_nodes], F16)     # X^T (prep scope only)

            h_chunks = {}
            for c in range(0, n_t, HCH):
                hc = hpool.tile([P, HCH * F], F16, tag=f"h16_{c}")
                nc.gpsimd.dma_start(
                    hc[:].rearrange("p (t f) -> p t f", f=F),
                    H_d[c * P:(c + HCH) * P, :].rearrange(
                        "(t p) f -> p t f", p=P))
                h_chunks[c] = hc

            # identity on gpsimd compute, then the first A emissions
            make_identity(nc, ident16[:])
            for _ in range(min(N_PRE, n_t)):
                load_a()

            QB = 4 if n_t % 4 == 0 else 2
            s_tiles = {}
            for t2 in range(0, n_t, QB):
                hT_ps = prep_ps.tile([P, QB * P], F16, tag="hT_ps")
                for k in range(QB):
                    t = t2 + k
                    hc = h_chunks[(t // HCH) * HCH]
                    nc.tensor.transpose(hT_ps[:, k * P:k * P + F],
                                        hc[:, (t % HCH) * F:(t % HCH + 1) * F],
                                        ident16[:])
                hT_sb = prep.tile([F, QB * P], F16)
                nc.scalar.copy(hT_sb[:], hT_ps[:F, 0:QB * P])
                # X^T tiles: [U, node QB*128]
                xT_ps = prep_ps.tile([U, QB * P], F32, tag="xps")
                nc.tensor.matmul(xT_ps[:], W_sb[:], hT_sb[:], start=True, stop=True)
                if (t2 // QB) % 2 == 0:
                    nc.scalar.copy(XT_sb[:, t2 * P:(t2 + QB) * P], xT_ps[:])
                else:
                    nc.vector.tensor_copy(XT_sb[:, t2 * P:(t2 + QB) * P], xT_ps[:])
                # s[p, t] = (X @ a1)[t*128+p]
                s_q = prep_ps1.tile([P, QB], F32, tag="s_q")
                for k in range(QB):
                    nc.tensor.matmul(s_q[:, k:k + 1],
                                     XT_sb[:, (t2 + k) * P:(t2 + k + 1) * P],
                                     a1_sb[:], start=True, stop=True)
                s_sb_q = persist.tile([P, QB], F32, tag=f"s{t2}")
                nc.vector.tensor_copy(s_sb_q[:], s_q[:])
                s_tiles[t2] = s_sb_q
                nc.vector.tensor_copy(s_sb[:, t2:t2 + QB], s_q[:])
                # n_bcast[p, slice] = n[slice] broadcast over partitions
                nb_ps = prep_ps.tile([P, QB * P], F32, tag="nb_ps")
                nc.tensor.matmul(nb_ps[:], a2b[:],
                                 XT_sb[:, t2 * P:(t2 + QB) * P],
                                 start=True, stop=True)
                nc.vector.tensor_copy(n_bcast[:, t2 * P:(t2 + QB) * P],
                                      nb_ps[:])
                # z_b = exp(0.2 n) straight from PSUM on ACT
                nc.scalar.activation(z_b[:, t2 * P:(t2 + QB) * P], nb_ps[:],
                                     AF.Exp, scale=LEAKY_SLOPE)

            # X tiles for the H_cap matmuls, rebuilt from X^T off the
            # critical path (overlaps the start of the main loop).
            for t in range(n_t):
                x_ps = prep_ps.tile([P, U], F16, tag="xps")
                nc.tensor.transpose(x_ps[:, 0:U],
                                    XT_sb[:, t * P:(t + 1) * P],
                                    ident16[0:U, 0:U])
                nc.vector.tensor_copy(Xp_sb[:, t * (U + 1):t * (U + 1) + U],
                                      x_ps[:])
            # w = exp(0.2 s) per-partition scalars
            nc.scalar.activation(w_sb[:], s_sb[:], AF.Exp, scale=LEAKY_SLOPE)

        # ---------------- main loop over query tiles ----------------
        p1set = set(i for i in P1_ITERS if i < n_t)
        GROUP = 16                     # transposes per PSUM tile (2 banks)
        n_groups = (n_t + GROUP - 1) // GROUP
        LOOK = 2                       # produce lookahead (iters)

        with tc.tile_pool(name="x1pool", bufs=2) as x1pool, \
             tc.tile_pool(name="x2pool", bufs=1) as x2pool, \
             tc.tile_pool(name="ppool", bufs=LOOK + 2) as ppool, \
             tc.tile_pool(name="ptpool", bufs=4) as ptpool, \
             tc.tile_pool(name="psT", bufs=3, space="PSUM") as psT, \
             tc.tile_pool(name="psAcc", bufs=2, space="PSUM") as psAcc:

            p_tiles = {}
            acc_tiles = {}

            def produce(it):
                load_a()               # keep the gpsimd queue fed, in order
                s_bias = s_tiles[(it // QB) * QB][:, it % QB:it % QB + 1]
                p_t = ppool.tile([P, n_nodes], F16, tag="p")
                if it in p1set:
                    # ACT-heavy: Prelu then Exp (both ScalarE, no DVE)
                    el = x1pool.tile([P, n_nodes], F16, tag="x1")
                    nc.scalar.activation(el[:], n_bcast[:], AF.Prelu,
                                         bias=s_bias, scale=1.0,
                                         alpha=LEAKY_SLOPE)
                    nc.scalar.activation(p_t[:], el[:], AF.Exp)
                else:
                    # DVE-heavy: x1 = exp(n+s) on ACT; x2 = z_b*w_i and
                    # p = max(x1, x2) on DVE
                    x1 = x1pool.tile([P, n_nodes], F16, tag="x1")
                    nc.scalar.activation(x1[:], n_bcast[:], AF.Exp, bias=s_bias)
                    x2 = x2pool.tile([P, n_nodes], F16, tag="x2")
                    nc.vector.tensor_scalar_mul(x2[:], z_b[:], w_sb[:, it:it + 1])
                    nc.vector.tensor_max(p_t[:], x1[:], x2[:])
                p_tiles[it] = p_t

            def consume(it):
                a_t = a_tiles.pop(it)
                p_t = p_tiles.pop(it)
                fine = it >= n_t - 2   # tail iterations: 8-block pipelining
                half = n_nodes // 2
                if not fine:
                    # mask in place on DVE, in halves so group transposes
                    # start earlier (never GpSimd: its tensor ops contend
                    # with DVE 2-port mode and slow everything down)
                    for hf in range(2):
                        nc.vector.tensor_mul(
                            p_t[:, hf * half:(hf + 1) * half],
                            p_t[:, hf * half:(hf + 1) * half],
                            a_t[:, hf * half:(hf + 1) * half])

                # transpose P_m 128x128 blocks -> PSUM, copy groups to SBUF
                acc_ps = psAcc.tile([P, U + 1], F32, tag="acc_ps")
                for g in range(n_groups):
                    k_n = min(GROUP, n_t - g * GROUP)
                    pt_ps = psT.tile([P, GROUP * P], F16, tag="pt_ps")
                    for half_g in range(2 if fine else 1):
                        if fine:
                            lo = g * GROUP * P + half_g * (GROUP // 2) * P
                            hi = lo + (GROUP // 2) * P
                            nc.vector.tensor_mul(p_t[:, lo:hi], p_t[:, lo:hi],
                                                 a_t[:, lo:hi])
                            ks = range(half_g * (GROUP // 2),
                                       min(k_n, (half_g + 1) * (GROUP // 2)))
                        else:
                            ks = range(k_n)
                        for k in ks:
                            jt = g * GROUP + k
                            nc.tensor.transpose(pt_ps[:, k * P:(k + 1) * P],
                                                p_t[:, jt * P:(jt + 1) * P],
                                                ident16[:])
                    pt_sb = ptpool.tile([P, GROUP * P], F16, tag="pt_sb")
                    w_n = k_n * P
                    if fine:
                        # split the copy across both engines in the tail
                        nc.scalar.copy(pt_sb[:, 0:w_n // 2], pt_ps[:, 0:w_n // 2])
                        nc.vector.tensor_copy(pt_sb[:, w_n // 2:w_n],
                                              pt_ps[:, w_n // 2:w_n])
                    else:
                        nc.vector.tensor_copy(pt_sb[:, 0:w_n], pt_ps[:, 0:w_n])
                    # H_cap accumulation for this group's j tiles
                    for k in range(k_n):
                        jt = g * GROUP + k
                        nc.tensor.matmul(
                            acc_ps[:], pt_sb[:, k * P:(k + 1) * P],
                            Xp_sb[:, jt * (U + 1):(jt + 1) * (U + 1)],
                            start=(jt == 0), stop=(jt == n_t - 1))

                nc.vector.reciprocal(dinv_sb[:, it:it + 1], acc_ps[:, U:U + 1])
                acc_tiles[it] = acc_ps

            def emit_out(it):
                # out = relu(H_cap[:, :U] / H_cap[:, U]) -- relu+scale on ACT,
                # into the SBUF output buffer (flushed by DMA at the end).
                acc_ps = acc_tiles.pop(it)
                nc.scalar.activation(outsbuf[:, it * U:(it + 1) * U],
                                     acc_ps[:, 0:U], AF.Relu,
                                     scale=dinv_sb[:, it:it + 1])

            for it in range(n_t + LOOK + 1):
                if it < n_t:
                    produce(it)
                if LOOK <= it < n_t + LOOK:
                    ct = it - LOOK
                    consume(ct)
                    if ct >= n_t - 2:
                        emit_out(ct)
                if LOOK < it < n_t + LOOK - 1:
                    emit_out(it - LOOK - 1)
                if FLUSH_SPLIT and it == n_t * 5 // 8:
                    hn = n_t // 2
                    nc.sync.dma_start(
                        out_d[0:hn * P, :].rearrange("(t p) u -> p t u", p=P),
                        outsbuf[:, 0:hn * U].rearrange("p (t u) -> p t u", u=U))

            # final output flush on the idle sync ring
            lo = (n_t // 2) if FLUSH_SPLIT else 0
            nc.sync.dma_start(
                out_d[lo * P:n_t * P, :].rearrange("(t p) u -> p t u", p=P),
                outsbuf[:, lo * U:n_t * U].rearrange("p (t u) -> p t u", u=U))

    nc.compile()
    return nc


_NC_CACHE = {}


def _get_nc(n_nodes=N_NODES):
    if n_nodes not in _NC_CACHE:
        _NC_CACHE[n_nodes] = build_nc(n_nodes)
    return _NC_CACHE[n_nodes]


def kernel(H, A, W, a_1, a_2):
    """Full inputs in, full output out. Shards batch across 8 NeuronCores."""
    import os
    # The axon trace path needs antenv.axon_hooks, which this image lacks;
    # make sure an inherited BASS_TRACE can't route us there.
    os.environ["BASS_NEVER_TRACE"] = "1"
    from concourse.bass_utils import run_bass_kernel_spmd

    B = H.shape[0]
    assert B == N_CORES
    nc = _get_nc(H.shape[1])
    in_maps = [
        {
            "H": np.ascontiguousarray(H[b], dtype=np.float32),
            "A": np.ascontiguousarray(A[b], dtype=np.float32),
            "W": np.ascontiguousarray(W, dtype=np.float32),
            "a_1": np.ascontiguousarray(a_1, dtype=np.float32),
            "a_2": np.ascontiguousarray(a_2, dtype=np.float32),
        }
        for b in range(B)
    ]
    res = run_bass_kernel_spmd(nc, in_maps, core_ids=list(range(N_CORES)))
    out = np.stack([res.results[b]["out"] for b in range(B)]).astype(np.float32)
    return out


# revision 7
# speedup vs baseline: 1.8082x; 1.1283x over previous
"""GAT-style graph attention kernel for Trainium2 (Bass/Tile), 8-core SPMD.

Per graph b (one NeuronCore each, B=8):
    X  = H[b] @ W                      [N, U]
    s  = X @ a_1   (per-query logit)   [N, 1]
    n  = X @ a_2   (per-key logit)     [N, 1]
    E  = leaky_relu(s_i + n_j, 0.2)    [N, N]
    P  = exp(E) * A[b]                 (== exp(E + NEG*(1-A)), A in {0,1})
    out= relu((P @ X) / rowsum(P))     [N, U]

v3 strategy (vs v2's two-frontend mix):
  Transport: 8-core microbenchmarks show ONE DMA queue with a deep
  buffer pool sustains ~375 GB/s while two concurrent queues interleave
  at packet granularity and drop to ~305.  So the whole A stream rides
  the gpsimd (SWDGE) queue as f32->f16-cast 2MiB singles with a
  12-buffer pool; W/a1/a2/H load f32 on the sync ring in the head
  (brief overlap only), and the outputs accumulate in SBUF and flush
  as one DMA at the very end.
  Compute: exp(leaky(s+n)) = max(exp(s+n), exp(0.2 s)*exp(0.2 n)).
  Per iteration: ONE ACT pass (x1 = Exp(n_bcast + s_i)), ONE fused DVE
  scalar_tensor_tensor pass (p = max(z_b * w_i, x1)), ONE DVE mask pass
  (p *= A, in place), plus the P^T PSUM->SBUF copies.  ACT ~3.7us,
  DVE ~3.6us, TensorE ~2.6us per iteration -- all under the ~5.6us
  DMA pace, so the kernel is memory-bound end to end and the compute
  tail after the last A byte is a single iteration's epilogue.
"""

import numpy as np
from contextlib import ExitStack

import concourse.bass as bass
import concourse.bacc as bacc
import concourse.mybir as mybir
import concourse.tile as tile
from concourse.masks import make_identity

F32 = mybir.dt.float32
F16 = mybir.dt.float16

N_NODES = 4096
N_FEAT = 128
N_UNITS = 64
N_CORES = 8
LEAKY_SLOPE = 0.2

A_BUFS = 11      # f16 A-tile pool depth (deep => SWDGE singles at line rate)
# ACT-heavy (Prelu+Exp, no DVE) iterations; the rest are DVE-heavy
# (Exp + tensor_scalar + max).  16:16 balances ACT ~5.8us vs DVE ~5.8us;
# iters 0-1 stay DVE-heavy (ACT is still finishing prep there).
P1_ITERS = (2, 4, 6, 8, 10, 12, 14, 15, 16, 18, 20, 21, 22, 24, 26, 28)
N_PRE = 4        # A loads issued before anything else on the gpsimd queue
FLUSH_SPLIT = True  # flush first half of outputs mid-stream


def build_nc(n_nodes=N_NODES):
    P = 128  # partitions
    U = N_UNITS
    F = N_FEAT
    n_t = n_nodes // P          # node tiles (32 full size)
    assert n_nodes % P == 0

    nc = bacc.Bacc(None)
    H_d = nc.declare_dram_parameter("H", [n_nodes, F], F32, isOutput=False)
    A_d = nc.declare_dram_parameter("A", [n_nodes, n_nodes], F32, isOutput=False)
    W_d = nc.declare_dram_parameter("W", [F, U], F32, isOutput=False)
    a1_d = nc.declare_dram_parameter("a_1", [U, 1], F32, isOutput=False)
    a2_d = nc.declare_dram_parameter("a_2", [U, 1], F32, isOutput=False)
    out_d = nc.declare_dram_parameter("out", [n_nodes, U], F32, isOutput=True)

    M = mybir.AluOpType
    AF = mybir.ActivationFunctionType

    with tile.TileContext(nc) as tc, ExitStack() as ctx:
        const = ctx.enter_context(tc.tile_pool(name="const", bufs=1))
        persist = ctx.enter_context(tc.tile_pool(name="persist", bufs=1))
        # A stream: deep f16 pool, one queue, strictly sequential issue.
        apool = ctx.enter_context(tc.tile_pool(name="apool", bufs=min(A_BUFS, n_t)))

        a_tiles = {}
        next_a = [0]

        def load_a():
            it = next_a[0]
            if it >= n_t:
                return
            next_a[0] = it + 1
            t = apool.tile([P, n_nodes], F16, tag="a16")
            nc.gpsimd.dma_start(t[:], A_d[it * P:(it + 1) * P, :])
            a_tiles[it] = t

        # Small weights + H ride the gpsimd queue (f32->f16 cast) AHEAD of
        # the A singles; a2 (kept f32) rides sync.
        W_sb = const.tile([F, U], F16)
        nc.gpsimd.dma_start(W_sb[:], W_d[:])
        a1_sb = const.tile([U, 1], F16)
        nc.gpsimd.dma_start(a1_sb[:], a1_d[:])
        a2_sb = const.tile([U, 1], F32)
        nc.sync.dma_start(a2_sb[:], a2_d[:])

        ident16 = const.tile([P, P], F16)

        # a2 broadcast along free dim: a2b[u, c] = a2[u]
        a2b = const.tile([U, P], F16)
        nc.vector.memset(a2b[:], 1.0)
        nc.vector.tensor_scalar_mul(a2b[:], a2b[:], a2_sb[:, 0:1])

        # persistent per-graph tensors
        n_bcast = persist.tile([P, n_nodes], F32)     # n[j] bcast over partitions
        z_b = persist.tile([P, n_nodes], F16)         # exp(0.2 n[j]) bcast
        Xp_sb = persist.tile([P, n_t * (U + 1)], F16)  # X' tiles [X_t | 1]
        s_sb = persist.tile([P, n_t], F32)            # s column per query tile
        w_sb = persist.tile([P, n_t], F32)            # exp(0.2 s)
        dinv_sb = persist.tile([P, n_t], F32)
        outsbuf = persist.tile([P, n_t * U], F32)     # all outputs, flushed late
        nc.vector.memset(Xp_sb[:], 1.0)

        HCH = max(1, n_t // 4)

        # ---------------- prep: X, X^T, s, z_b, n_bcast ----------------
        with tc.tile_pool(name="hpool", bufs=1) as hpool, \
             tc.tile_pool(name="prep", bufs=6) as prep, \
             tc.tile_pool(name="prepx", bufs=1) as prepx, \
             tc.tile_pool(name="prep_ps", bufs=2, space="PSUM") as prep_ps, \
             tc.tile_pool(name="prep_ps1", bufs=2, space="PSUM") as prep_ps1:

            XT_sb = prepx.tile([U, n_nodes], F16)     # X^T (prep scope only)

            h_chunks = {}
            for c in range(0, n_t, HCH):
                hc = hpool.tile([P, HCH * F], F16, tag=f"h16_{c}")
                nc.gpsimd.dma_start(
                    hc[:].rearrange("p (t f) -> p t f", f=F),
                    H_d[c * P:(c + HCH) * P, :].rearrange(
                        "(t p) f -> p t f", p=P))
                h_chunks[c] = hc

            # identity on gpsimd compute, then the first A emissions
            make_identity(nc, ident16[:])
            for _ in range(min(N_PRE, n_t)):
                load_a()

            QB = 4 if n_t % 4 == 0 else 2
            s_tiles = {}
            for t2 in range(0, n_t, QB):
                hT_ps = prep_ps.tile([P, QB * P], F16, tag="hT_ps")
                for k in range(QB):
                    t = t2 + k
                    hc = h_chunks[(t // HCH) * HCH]
                    nc.tensor.transpose(hT_ps[:, k * P:k * P + F],
                                        hc[:, (t % HCH) * F:(t % HCH + 1) * F],
                                        ident16[:])
                hT_sb = prep.tile([F, QB * P], F16)
                nc.scalar.copy(hT_sb[:], hT_ps[:F, 0:QB * P])
                # X^T tiles: [U, node QB*128]
                xT_ps = prep_ps.tile([U, QB * P], F32, tag="xps")
                nc.tensor.matmul(xT_ps[:], W_sb[:], hT_sb[:], start=True, stop=True)
                if (t2 // QB) % 2 == 0:
                    nc.scalar.copy(XT_sb[:, t2 * P:(t2 + QB) * P], xT_ps[:])
                else:
                    nc.vector.tensor_copy(XT_sb[:, t2 * P:(t2 + QB) * P], xT_ps[:])
                # s[p, t] = (X @ a1)[t*128+p]
                s_q = prep_ps1.tile([P, QB], F32, tag="s_q")
                for k in range(QB):
                    nc.tensor.matmul(s_q[:, k:k + 1],
                                     XT_sb[:, (t2 + k) * P:(t2 + k + 1) * P],
                                     a1_sb[:], start=True, stop=True)
                s_sb_q = persist.tile([P, QB], F32, tag=f"s{t2}")
                nc.vector.tensor_copy(s_sb_q[:], s_q[:])
                s_tiles[t2] = s_sb_q
                nc.vector.tensor_copy(s_sb[:, t2:t2 + QB], s_q[:])
                # n_bcast[p, slice] = n[slice] broadcast over partitions
                nb_ps = prep_ps.tile([P, QB * P], F32, tag="nb_ps")
                nc.tensor.matmul(nb_ps[:], a2b[:],
                                 XT_sb[:, t2 * P:(t2 + QB) * P],
                                 start=True, stop=True)
                nc.vector.tensor_copy(n_bcast[:, t2 * P:(t2 + QB) * P],
                                      nb_ps[:])
                # z_b = exp(0.2 n) straight from PSUM on ACT
                nc.scalar.activation(z_b[:, t2 * P:(t2 + QB) * P], nb_ps[:],
                                     AF.Exp, scale=LEAKY_SLOPE)

            # X tiles for the H_cap matmuls, rebuilt from X^T off the
            # critical path (overlaps the start of the main loop).
            for t in range(n_t):
                x_ps = prep_ps.tile([P, U], F16, tag="xps")
                nc.tensor.transpose(x_ps[:, 0:U],
                                    XT_sb[:, t * P:(t + 1) * P],
                                    ident16[0:U, 0:U])
                nc.vector.tensor_copy(Xp_sb[:, t * (U + 1):t * (U + 1) + U],
                                      x_ps[:])
            # w = exp(0.2 s) per-partition scalars
            nc.scalar.activation(w_sb[:], s_sb[:], AF.Exp, scale=LEAKY_SLOPE)

        # ---------------- main loop over query tiles ----------------
        p1set = set(i for i in P1_ITERS if i < n_t)
        GROUP = 16                     # transposes per PSUM tile (2 banks)
        n_groups = (n_t + GROUP - 1) // GROUP
        LOOK = 2                       # produce lookahead (iters)

        with tc.tile_pool(name="x1pool", bufs=2) as x1pool, \
             tc.tile_pool(name="x2pool", bufs=1) as x2pool, \
             tc.tile_pool(name="ppool", bufs=LOOK + 2) as ppool, \
             tc.tile_pool(name="ptpool", bufs=4) as ptpool, \
             tc.tile_pool(name="psT", bufs=3, space="PSUM") as psT, \
             tc.tile_pool(name="psAcc", bufs=2, space="PSUM") as psAcc:

            p_tiles = {}
            acc_tiles = {}

            def produce(it):
                load_a()               # keep the gpsimd queue fed, in order
                s_bias = s_tiles[(it // QB) * QB][:, it % QB:it % QB + 1]
                p_t = ppool.tile([P, n_nodes], F16, tag="p")
                if it in p1set:
                    # ACT-heavy: Prelu then Exp (both ScalarE, no DVE)
                    el = x1pool.tile([P, n_nodes], F16, tag="x1")
                    nc.scalar.activation(el[:], n_bcast[:], AF.Prelu,
                                         bias=s_bias, scale=1.0,
                                         alpha=LEAKY_SLOPE)
                    nc.scalar.activation(p_t[:], el[:], AF.Exp)
                else:
                    # DVE-heavy: x1 = exp(n+s) on ACT; x2 = z_b*w_i and
                    # p = max(x1, x2) on DVE
                    x1 = x1pool.tile([P, n_nodes], F16, tag="x1")
                    nc.scalar.activation(x1[:], n_bcast[:], AF.Exp, bias=s_bias)
                    x2 = x2pool.tile([P, n_nodes], F16, tag="x2")
                    nc.vector.tensor_scalar_mul(x2[:], z_b[:], w_sb[:, it:it + 1])
                    nc.vector.tensor_max(p_t[:], x1[:], x2[:])
                p_tiles[it] = p_t

            def consume(it):
                a_t = a_tiles.pop(it)
                p_t = p_tiles.pop(it)
                fine = it >= n_t - 2   # tail iterations: 8-block pipelining
                half = n_nodes // 2
                if not fine:
                    # mask in place on DVE, in halves so group transposes
                    # start earlier (never GpSimd: its tensor ops contend
                    # with DVE 2-port mode and slow everything down)
                    for hf in range(2):
                        nc.vector.tensor_mul(
                            p_t[:, hf * half:(hf + 1) * half],
                            p_t[:, hf * half:(hf + 1) * half],
                            a_t[:, hf * half:(hf + 1) * half])

                # transpose P_m 128x128 blocks -> PSUM, copy groups to SBUF
                acc_ps = psAcc.tile([P, U + 1], F32, tag="acc_ps")
                for g in range(n_groups):
                    k_n = min(GROUP, n_t - g * GROUP)
                    pt_ps = psT.tile([P, GROUP * P], F16, tag="pt_ps")
                    for half_g in range(2 if fine else 1):
                        if fine:
                            lo = g * GROUP * P + half_g * (GROUP // 2) * P
                            hi = lo + (GROUP // 2) * P
                            nc.vector.tensor_mul(p_t[:, lo:hi], p_t[:, lo:hi],
                                                 a_t[:, lo:hi])
                            ks = range(half_g * (GROUP // 2),
                                       min(k_n, (half_g + 1) * (GROUP // 2)))
                        else:
                            ks = range(k_n)
                        for k in ks:
                            jt = g * GROUP + k
                            nc.tensor.transpose(pt_ps[:, k * P:(k + 1) * P],
                                                p_t[:, jt * P:(jt + 1) * P],
                                                ident16[:])
                    pt_sb = ptpool.tile([P, GROUP * P], F16, tag="pt_sb")
                    w_n = k_n * P
                    if fine:
                        # split the copy across both engines in the tail
                        nc.scalar.copy(pt_sb[:, 0:w_n // 2], pt_ps[:, 0:w_n // 2])
                        nc.vector.tensor_copy(pt_sb[:, w_n // 2:w_n],
                                              pt_ps[:, w_n // 2:w_n])
                    else:
                        nc.vector.tensor_copy(pt_sb[:, 0:w_n], pt_ps[:, 0:w_n])
                    # H_cap accumulation for this group's j tiles
                    for k in range(k_n):
                        jt = g * GROUP + k
                        nc.tensor.matmul(
                            acc_ps[:], pt_sb[:, k * P:(k + 1) * P],
                            Xp_sb[:, jt * (U + 1):(jt + 1) * (U + 1)],
                            start=(jt == 0), stop=(jt == n_t - 1))

                nc.vector.reciprocal(dinv_sb[:, it:it + 1], acc_ps[:, U:U + 1])
                acc_tiles[it] = acc_ps

            def emit_out(it):
                # out = relu(H_cap[:, :U] / H_cap[:, U]) -- relu+scale on ACT,
                # into the SBUF output buffer (flushed by DMA at the end).
                acc_ps = acc_tiles.pop(it)
                nc.scalar.activation(outsbuf[:, it * U:(it + 1) * U],
                                     acc_ps[:, 0:U], AF.Relu,
                                     scale=dinv_sb[:, it:it + 1])

            for it in range(n_t + LOOK + 1):
                if it < n_t:
                    produce(it)
                if LOOK <= it < n_t + LOOK:
                    ct = it - LOOK
                    consume(ct)
                    if ct >= n_t - 2:
                        emit_out(ct)
                if LOOK < it < n_t + LOOK - 1:
                    emit_out(it - LOOK - 1)
                if FLUSH_SPLIT and it == n_t * 5 // 8:
                    hn = n_t // 2
                    nc.sync.dma_start(
                        out_d[0:hn * P, :].rearrange("(t p) u -> p t u", p=P),
                        outsbuf[:, 0:hn * U].rearrange("p (t u) -> p t u", u=U))

            # final output flush on the idle sync ring
            lo = (n_t // 2) if FLUSH_SPLIT else 0
            nc.sync.dma_start(
                out_d[lo * P:n_t * P, :].rearrange("(t p) u -> p t u", p=P),
                outsbuf[:, lo * U:n_t * U].rearrange("p (t u) -> p t u", u=U))

    nc.compile()
    return nc


_NC_CACHE = {}


def _get_nc(n_nodes=N_NODES):
    if n_nodes not in _NC_CACHE:
        _NC_CACHE[n_nodes] = build_nc(n_nodes)
    return _NC_CACHE[n_nodes]


def kernel(H, A, W, a_1, a_2):
    """Full inputs in, full output out. Shards batch across 8 NeuronCores."""
    import os
    # The axon trace path needs antenv.axon_hooks, which this image lacks;
    # make sure an inherited BASS_TRACE can't route us there.
    os.environ["BASS_NEVER_TRACE"] = "1"
    from concourse.bass_utils import run_bass_kernel_spmd

    B = H.shape[0]
    assert B == N_CORES
    nc = _get_nc(H.shape[1])
    in_maps = [
        {
            "H": np.ascontiguousarray(H[b], dtype=np.float32),
            "A": np.ascontiguousarray(A[b], dtype=np.float32),
            "W": np.ascontiguousarray(W, dtype=np.float32),
            "a_1": np.ascontiguousarray(a_1, dtype=np.float32),
            "a_2": np.ascontiguousarray(a_2, dtype=np.float32),
        }
        for b in range(B)
    ]
    res = run_bass_kernel_spmd(nc, in_maps, core_ids=list(range(N_CORES)))
    out = np.stack([res.results[b]["out"] for b in range(B)]).astype(np.float32)
    return out


# revision 8
# speedup vs baseline: 1.9570x; 1.0823x over previous
"""GAT-style graph attention kernel for Trainium2 (Bass/Tile), 8-core SPMD.

Per graph b (one NeuronCore each, B=8):
    X  = H[b] @ W                      [N, U]
    s  = X @ a_1   (per-query logit)   [N, 1]
    n  = X @ a_2   (per-key logit)     [N, 1]
    E  = leaky_relu(s_i + n_j, 0.2)    [N, N]
    P  = exp(E) * A[b]                 (== exp(E + NEG*(1-A)), A in {0,1})
    out= relu((P @ X) / rowsum(P))     [N, U]

v3 strategy (vs v2's two-frontend mix):
  Transport: 8-core microbenchmarks show ONE DMA queue with a deep
  buffer pool sustains ~375 GB/s while two concurrent queues interleave
  at packet granularity and drop to ~305.  So the whole A stream rides
  the gpsimd (SWDGE) queue as f32->f16-cast 2MiB singles with a
  12-buffer pool; W/a1/a2/H load f32 on the sync ring in the head
  (brief overlap only), and the outputs accumulate in SBUF and flush
  as one DMA at the very end.
  Compute: exp(leaky(s+n)) = max(exp(s+n), exp(0.2 s)*exp(0.2 n)).
  Per iteration: ONE ACT pass (x1 = Exp(n_bcast + s_i)), ONE fused DVE
  scalar_tensor_tensor pass (p = max(z_b * w_i, x1)), ONE DVE mask pass
  (p *= A, in place), plus the P^T PSUM->SBUF copies.  ACT ~3.7us,
  DVE ~3.6us, TensorE ~2.6us per iteration -- all under the ~5.6us
  DMA pace, so the kernel is memory-bound end to end and the compute
  tail after the last A byte is a single iteration's epilogue.
"""

import numpy as np
from contextlib import ExitStack

import concourse.bass as bass
import concourse.bacc as bacc
import concourse.mybir as mybir
import concourse.tile as tile
from concourse.masks import make_identity

F32 = mybir.dt.float32
F16 = mybir.dt.float16

N_NODES = 4096
N_FEAT = 128
N_UNITS = 64
N_CORES = 8
LEAKY_SLOPE = 0.2

A_BUFS = 10      # f16 A-tile pool depth (deep => SWDGE singles at line rate)
# P5 iterations produce p purely on DVE via rank-1 factors:
#   x1 = exp(s_i)*exp(n_j), x2 = exp(0.2 s_i)*exp(0.2 n_j), p = max(x1,x2)
# (two tensor_scalar + one max).  All other iterations are ACT-heavy P1
# (Prelu then Exp).  22:10 balances ACT ~5.3us vs DVE ~5.5us per iter.
P5_ITERS = (0, 3, 6, 10, 13, 16, 20, 23, 26, 29)
N_PRE = 4        # A loads issued before anything else on the gpsimd queue
FLUSH_SPLIT = True  # flush first half of outputs mid-stream


def build_nc(n_nodes=N_NODES):
    P = 128  # partitions
    U = N_UNITS
    F = N_FEAT
    n_t = n_nodes // P          # node tiles (32 full size)
    assert n_nodes % P == 0

    nc = bacc.Bacc(None)
    H_d = nc.declare_dram_parameter("H", [n_nodes, F], F32, isOutput=False)
    A_d = nc.declare_dram_parameter("A", [n_nodes, n_nodes], F32, isOutput=False)
    W_d = nc.declare_dram_parameter("W", [F, U], F32, isOutput=False)
    a1_d = nc.declare_dram_parameter("a_1", [U, 1], F32, isOutput=False)
    a2_d = nc.declare_dram_parameter("a_2", [U, 1], F32, isOutput=False)
    out_d = nc.declare_dram_parameter("out", [n_nodes, U], F32, isOutput=True)

    M = mybir.AluOpType
    AF = mybir.ActivationFunctionType

    with tile.TileContext(nc) as tc, ExitStack() as ctx:
        const = ctx.enter_context(tc.tile_pool(name="const", bufs=1))
        persist = ctx.enter_context(tc.tile_pool(name="persist", bufs=1))
        # A stream: deep f16 pool, one queue, strictly sequential issue.
        apool = ctx.enter_context(tc.tile_pool(name="apool", bufs=min(A_BUFS, n_t)))

        a_tiles = {}
        next_a = [0]

        def load_a():
            it = next_a[0]
            if it >= n_t:
                return
            next_a[0] = it + 1
            t = apool.tile([P, n_nodes], F16, tag="a16")
            nc.gpsimd.dma_start(t[:], A_d[it * P:(it + 1) * P, :])
            a_tiles[it] = t

        # Small weights + H ride the gpsimd queue (f32->f16 cast) AHEAD of
        # the A singles; a2 (kept f32) rides sync.
        W_sb = const.tile([F, U], F16)
        nc.gpsimd.dma_start(W_sb[:], W_d[:])
        a1_sb = const.tile([U, 1], F16)
        nc.gpsimd.dma_start(a1_sb[:], a1_d[:])
        a2_sb = const.tile([U, 1], F32)
        nc.sync.dma_start(a2_sb[:], a2_d[:])

        ident16 = const.tile([P, P], F16)

        # a2 broadcast along free dim: a2b[u, c] = a2[u]
        a2b = const.tile([U, P], F16)
        nc.vector.memset(a2b[:], 1.0)
        nc.vector.tensor_scalar_mul(a2b[:], a2b[:], a2_sb[:, 0:1])

        # persistent per-graph tensors
        n_bcast = persist.tile([P, n_nodes], F32)     # n[j] bcast over partitions
        z_b = persist.tile([P, n_nodes], F16)         # exp(0.2 n[j]) bcast
        en_b = persist.tile([P, n_nodes], F16)        # exp(n[j]) bcast
        Xp_sb = persist.tile([P, n_t * (U + 1)], F16)  # X' tiles [X_t | 1]
        s_sb = persist.tile([P, n_t], F32)            # s column per query tile
        w_sb = persist.tile([P, n_t], F32)            # exp(0.2 s)
        es_sb = persist.tile([P, n_t], F32)           # exp(s)
        dinv_sb = persist.tile([P, n_t], F32)
        outsbuf = persist.tile([P, n_t * U], F32)     # all outputs, flushed late
        nc.vector.memset(Xp_sb[:], 1.0)

        HCH = max(1, n_t // 4)

        # ---------------- prep: X, X^T, s, z_b, n_bcast ----------------
        with tc.tile_pool(name="hpool", bufs=1) as hpool, \
             tc.tile_pool(name="prep", bufs=6) as prep, \
             tc.tile_pool(name="prepx", bufs=1) as prepx, \
             tc.tile_pool(name="prep_ps", bufs=2, space="PSUM") as prep_ps, \
             tc.tile_pool(name="prep_ps1", bufs=2, space="PSUM") as prep_ps1:

            XT_sb = prepx.tile([U, n_nodes], F16)     # X^T (prep scope only)

            h_chunks = {}
            for c in range(0, n_t, HCH):
                hc = hpool.tile([P, HCH * F], F16, tag=f"h16_{c}")
                nc.gpsimd.dma_start(
                    hc[:].rearrange("p (t f) -> p t f", f=F),
                    H_d[c * P:(c + HCH) * P, :].rearrange(
                        "(t p) f -> p t f", p=P))
                h_chunks[c] = hc

            # identity on gpsimd compute, then the first A emissions
            make_identity(nc, ident16[:])
            for _ in range(min(N_PRE, n_t)):
                load_a()

            QB = 4 if n_t % 4 == 0 else 2
            s_tiles = {}
            for t2 in range(0, n_t, QB):
                hT_ps = prep_ps.tile([P, QB * P], F16, tag="hT_ps")
                for k in range(QB):
                    t = t2 + k
                    hc = h_chunks[(t // HCH) * HCH]
                    nc.tensor.transpose(hT_ps[:, k * P:k * P + F],
                                        hc[:, (t % HCH) * F:(t % HCH + 1) * F],
                                        ident16[:])
                hT_sb = prep.tile([F, QB * P], F16)
                nc.scalar.copy(hT_sb[:], hT_ps[:F, 0:QB * P])
                # X^T tiles: [U, node QB*128]
                xT_ps = prep_ps.tile([U, QB * P], F32, tag="xps")
                nc.tensor.matmul(xT_ps[:], W_sb[:], hT_sb[:], start=True, stop=True)
                if (t2 // QB) % 2 == 0:
                    nc.scalar.copy(XT_sb[:, t2 * P:(t2 + QB) * P], xT_ps[:])
                else:
                    nc.vector.tensor_copy(XT_sb[:, t2 * P:(t2 + QB) * P], xT_ps[:])
                # s[p, t] = (X @ a1)[t*128+p]
                s_q = prep_ps1.tile([P, QB], F32, tag="s_q")
                for k in range(QB):
                    nc.tensor.matmul(s_q[:, k:k + 1],
                                     XT_sb[:, (t2 + k) * P:(t2 + k + 1) * P],
                                     a1_sb[:], start=True, stop=True)
                s_sb_q = persist.tile([P, QB], F32, tag=f"s{t2}")
                nc.vector.tensor_copy(s_sb_q[:], s_q[:])
                s_tiles[t2] = s_sb_q
                nc.vector.tensor_copy(s_sb[:, t2:t2 + QB], s_q[:])
                # n_bcast[p, slice] = n[slice] broadcast over partitions
                nb_ps = prep_ps.tile([P, QB * P], F32, tag="nb_ps")
                nc.tensor.matmul(nb_ps[:], a2b[:],
                                 XT_sb[:, t2 * P:(t2 + QB) * P],
                                 start=True, stop=True)
                nc.vector.tensor_copy(n_bcast[:, t2 * P:(t2 + QB) * P],
                                      nb_ps[:])
                # z_b = exp(0.2 n), en_b = exp(n) straight from PSUM on ACT
                nc.scalar.activation(z_b[:, t2 * P:(t2 + QB) * P], nb_ps[:],
                                     AF.Exp, scale=LEAKY_SLOPE)
                nc.scalar.activation(en_b[:, t2 * P:(t2 + QB) * P], nb_ps[:],
                                     AF.Exp)

            # X tiles for the H_cap matmuls, rebuilt from X^T off the
            # critical path (overlaps the start of the main loop).
            for t in range(n_t):
                x_ps = prep_ps.tile([P, U], F16, tag="xps")
                nc.tensor.transpose(x_ps[:, 0:U],
                                    XT_sb[:, t * P:(t + 1) * P],
                                    ident16[0:U, 0:U])
                nc.vector.tensor_copy(Xp_sb[:, t * (U + 1):t * (U + 1) + U],
                                      x_ps[:])
            # w = exp(0.2 s), es = exp(s) per-partition scalars
            nc.scalar.activation(w_sb[:], s_sb[:], AF.Exp, scale=LEAKY_SLOPE)
            nc.scalar.activation(es_sb[:], s_sb[:], AF.Exp)

        # ---------------- main loop over query tiles ----------------
        p5set = set(i for i in P5_ITERS if i < n_t)
        GROUP = 16                     # transposes per PSUM tile (2 banks)
        n_groups = (n_t + GROUP - 1) // GROUP
        LOOK = 2                       # produce lookahead (iters)

        with tc.tile_pool(name="x1pool", bufs=2) as x1pool, \
             tc.tile_pool(name="x2pool", bufs=1) as x2pool, \
             tc.tile_pool(name="ppool", bufs=LOOK + 2) as ppool, \
             tc.tile_pool(name="ptpool", bufs=4) as ptpool, \
             tc.tile_pool(name="psT", bufs=3, space="PSUM") as psT, \
             tc.tile_pool(name="psAcc", bufs=2, space="PSUM") as psAcc:

            p_tiles = {}
            acc_tiles = {}

            def produce(it):
                load_a()               # keep the gpsimd queue fed, in order
                s_bias = s_tiles[(it // QB) * QB][:, it % QB:it % QB + 1]
                p_t = ppool.tile([P, n_nodes], F16, tag="p")
                if it in p5set:
                    # pure-DVE: x1 = es_i*en_j, x2 = w_i*z_j, p = max
                    x1 = x1pool.tile([P, n_nodes], F16, tag="x1")
                    nc.vector.tensor_scalar_mul(x1[:], en_b[:],
                                                es_sb[:, it:it + 1])
                    x2 = x2pool.tile([P, n_nodes], F16, tag="x2")
                    nc.vector.tensor_scalar_mul(x2[:], z_b[:], w_sb[:, it:it + 1])
                    nc.vector.tensor_max(p_t[:], x1[:], x2[:])
                else:
                    # ACT-heavy: Prelu then Exp (both ScalarE, no DVE)
                    el = x1pool.tile([P, n_nodes], F16, tag="x1")
                    nc.scalar.activation(el[:], n_bcast[:], AF.Prelu,
                                         bias=s_bias, scale=1.0,
                                         alpha=LEAKY_SLOPE)
                    nc.scalar.activation(p_t[:], el[:], AF.Exp)
                p_tiles[it] = p_t

            def consume(it):
                a_t = a_tiles.pop(it)
                p_t = p_tiles.pop(it)
                fine = it >= n_t - 2   # tail iterations: 8-block pipelining
                half = n_nodes // 2
                if not fine:
                    # mask in place on DVE, one full pass (fewer drains;
                    # never GpSimd: its tensor ops contend with DVE 2-port
                    # mode and slow everything down)
                    nc.vector.tensor_mul(p_t[:], p_t[:], a_t[:])

                # transpose P_m 128x128 blocks -> PSUM, copy groups to SBUF
                acc_ps = psAcc.tile([P, U + 1], F32, tag="acc_ps")
                for g in range(n_groups):
                    k_n = min(GROUP, n_t - g * GROUP)
                    pt_ps = psT.tile([P, GROUP * P], F16, tag="pt_ps")
                    for half_g in range(2 if fine else 1):
                        if fine:
                            lo = g * GROUP * P + half_g * (GROUP // 2) * P
                            hi = lo + (GROUP // 2) * P
                            nc.vector.tensor_mul(p_t[:, lo:hi], p_t[:, lo:hi],
                                                 a_t[:, lo:hi])
                            ks = range(half_g * (GROUP // 2),
                                       min(k_n, (half_g + 1) * (GROUP // 2)))
                        else:
                            ks = range(k_n)
                        for k in ks:
                            jt = g * GROUP + k
                            nc.tensor.transpose(pt_ps[:, k * P:(k + 1) * P],
                                                p_t[:, jt * P:(jt + 1) * P],
                                                ident16[:])
                    pt_sb = ptpool.tile([P, GROUP * P], F16, tag="pt_sb")
                    w_n = k_n * P
                    if fine:
                        # split the copy across both engines in the tail
                        nc.scalar.copy(pt_sb[:, 0:w_n // 2], pt_ps[:, 0:w_n // 2])
                        nc.vector.tensor_copy(pt_sb[:, w_n // 2:w_n],
                                              pt_ps[:, w_n // 2:w_n])
                    else:
                        nc.vector.tensor_copy(pt_sb[:, 0:w_n], pt_ps[:, 0:w_n])
                    # H_cap accumulation for this group's j tiles
                    for k in range(k_n):
                        jt = g * GROUP + k
                        nc.tensor.matmul(
                            acc_ps[:], pt_sb[:, k * P:(k + 1) * P],
                            Xp_sb[:, jt * (U + 1):(jt + 1) * (U + 1)],
                            start=(jt == 0), stop=(jt == n_t - 1))

                nc.vector.reciprocal(dinv_sb[:, it:it + 1], acc_ps[:, U:U + 1])
                acc_tiles[it] = acc_ps

            def emit_out(it):
                # out = relu(H_cap[:, :U] / H_cap[:, U]) -- relu+scale on ACT,
                # into the SBUF output buffer (flushed by DMA at the end).
                acc_ps = acc_tiles.pop(it)
                nc.scalar.activation(outsbuf[:, it * U:(it + 1) * U],
                                     acc_ps[:, 0:U], AF.Relu,
                                     scale=dinv_sb[:, it:it + 1])

            for it in range(n_t + LOOK + 1):
                if it < n_t:
                    produce(it)
                if LOOK <= it < n_t + LOOK:
                    ct = it - LOOK
                    consume(ct)
                    if ct >= n_t - 2:
                        emit_out(ct)
                if LOOK < it < n_t + LOOK - 1:
                    emit_out(it - LOOK - 1)
                if FLUSH_SPLIT and it == n_t * 5 // 8:
                    hn = n_t // 2
                    nc.sync.dma_start(
                        out_d[0:hn * P, :].rearrange("(t p) u -> p t u", p=P),
                        outsbuf[:, 0:hn * U].rearrange("p (t u) -> p t u", u=U))

            # final output flush on the idle sync ring
            lo = (n_t // 2) if FLUSH_SPLIT else 0
            nc.sync.dma_start(
                out_d[lo * P:n_t * P, :].rearrange("(t p) u -> p t u", p=P),
                outsbuf[:, lo * U:n_t * U].rearrange("p (t u) -> p t u", u=U))

    nc.compile()
    return nc


_NC_CACHE = {}


def _get_nc(n_nodes=N_NODES):
    if n_nodes not in _NC_CACHE:
        _NC_CACHE[n_nodes] = build_nc(n_nodes)
    return _NC_CACHE[n_nodes]


def kernel(H, A, W, a_1, a_2):
    """Full inputs in, full output out. Shards batch across 8 NeuronCores."""
    import os
    # The axon trace path needs antenv.axon_hooks, which this image lacks;
    # make sure an inherited BASS_TRACE can't route us there.
    os.environ["BASS_NEVER_TRACE"] = "1"
    from concourse.bass_utils import run_bass_kernel_spmd

    B = H.shape[0]
    assert B == N_CORES
    nc = _get_nc(H.shape[1])
    in_maps = [
        {
            "H": np.ascontiguousarray(H[b], dtype=np.float32),
            "A": np.ascontiguousarray(A[b], dtype=np.float32),
            "W": np.ascontiguousarray(W, dtype=np.float32),
            "a_1": np.ascontiguousarray(a_1, dtype=np.float32),
            "a_2": np.ascontiguousarray(a_2, dtype=np.float32),
        }
        for b in range(B)
    ]
    res = run_bass_kernel_spmd(nc, in_maps, core_ids=list(range(N_CORES)))
    out = np.stack([res.results[b]["out"] for b in range(B)]).astype(np.float32)
    return out
